# revision 35
# baseline (speedup 1.0000x reference)
"""BSplineKAN layer forward on 8 Trainium2 NeuronCores (Bass/Tile).

out = silu(x @ base_weight) + einsum('bir,ior->bo', bspline_basis(x), coeff)

Math: with uniform knots t_j = t3 + (j-3)*h (t3 = left clamp bound) and
s = clamp(inv_h*x + s_bias, 0, S) in [0, N_GRID], the cubic B-spline basis is
an exact linear combination of centered/short-side truncated-power features
    {c, c^2, c^3, (1-s)+^3, (2-s)+^3, (s-3)+^3, (s-4)+^3, 1},  c = s - 2.5.
(Short-side cubes and the centered cubic keep |feature| <= 16, vs 125 for
plain {s,s^2,s^3,(s-k)+^3} — 6x less f16 cancellation error.)
The (features -> basis) map M is solved on the host in float64 and folded into
coeff:  W2[(f,i), o] = sum_r M[f,r] * coeff[i,o,r].  The constant feature is
shared across i, so it collapses to a single bias row (K=1 matmul with a ones
vector) instead of 512 K-rows — the spline contraction is 7*512+1 instead of
8*512.

Device schedule per 512-row window (software pipelined):
  PE:  4x4 [128,128] f32 transposes -> pt (PSUM), then 29+4 f16 matmuls/tile
  ACT: xth = Copy(pt) [base input], sr = Relu(inv_h*pt+s_bias) [clamp left],
       c2 = Square(c), u_k = Relu(+-c + b_k), silu epilogue
  DVE: c = min(sr,S)-2.5 [clamp right+center], c3 = c2*c, u2 = u*u, L = u2*u
       (all-f16 tensor_tensor ops run in 2x DVE mode)
  Pool: out = acc_s + silu (PSUM drain)
Transposes for window w+1 are emitted before the matmuls of window w so the
feature chain of w+1 (ACT/DVE) overlaps the matmuls of w (PE).

Sharding: data-parallel over batch; each of 8 cores handles 2048 rows with
replicated weights. No collectives.
"""

import numpy as np

import concourse.bass as bass
import concourse.mybir as mybir
import concourse.tile as tile
from concourse import bacc
from concourse.bass_utils import run_bass_kernel_spmd
from concourse.masks import make_identity

N_CORES = 8
BATCH, N_IN, N_OUT = 16384, 512, 512
SPLINE_ORDER, N_GRID = 3, 5
N_BASIS = N_GRID + SPLINE_ORDER  # 8
B_CORE = BATCH // N_CORES        # 2048
N_IC = N_IN // 128               # 4 contraction chunks per feature
N_FEAT = 7                       # c, c^2, c^3, short-side cubes k=1..4
N_W2_ROWS = N_FEAT * N_IN + 1    # + shared const bias row
# window sizes (batch rows): two small startup windows shorten the time to
# the first matmul; 512-row windows amortize per-op overhead afterwards.
WINDOWS = [256, 256, 512, 512, 512]
assert sum(WINDOWS) == B_CORE

f32 = mybir.dt.float32
f16 = mybir.dt.float16
AF = mybir.ActivationFunctionType
ALU = mybir.AluOpType

# which engine handles the epilogue add (gpsimd/Pool cannot read PSUM)
EPI_ENGINE = "dve"


# ----------------------------------------------------------------------------
# Host-side math
# ----------------------------------------------------------------------------

def _bspline_basis_f64(x, knots):
    """Cox-de Boor recursion (float64), matching the reference semantics."""
    t = np.asarray(knots, np.float64)
    xc = np.clip(np.asarray(x, np.float64),
                 t[SPLINE_ORDER], t[-SPLINE_ORDER - 1])[..., None]
    n_int = len(t) - 1
    B = ((xc >= t[:-1]) & (xc < t[1:])).astype(np.float64)
    for j in range(1, SPLINE_ORDER + 1):
        nv = n_int - j
        ti = t[:nv]
        ti_j = t[j:nv + j]
        ti1 = t[1:nv + 1]
        ti_j1 = t[j + 1:nv + j + 1]
        a1 = (xc - ti) / np.maximum(ti_j - ti, 1e-8)
        a2 = (ti_j1 - xc) / np.maximum(ti_j1 - ti1, 1e-8)
        B = a1 * B[..., :nv] + a2 * B[..., 1:nv + 1]
    return B  # (..., N_BASIS)


# (knot, sign): sign -1 -> (k-s)+^3 [short side left], +1 -> (s-k)+^3
CUBES = [(1, -1.0), (2, -1.0), (3, 1.0), (4, 1.0)]
C_CENTER = 2.5
# all cube relus run on DVE (tensor_scalar min/max of c):
#   sgn=+1: u = (c - (k-C)) max 0 = relu(s-k)          -> feature = +cube
#   sgn=-1: m = (c + (C-k)) min 0 = -relu(k-s)  (m^3 = -cube) -> w2 negated
DVE_CUBES = (0, 1, 2, 3)


def _features_f64(s):
    """Centered/short-side features of s (float64): (..., 8) incl. const."""
    c = s - C_CENTER
    F = [c, c * c, c ** 3]
    for k, sgn in CUBES:
        u = np.maximum(sgn * (s - k), 0.0)
        F.append(u ** 3)
    F.append(np.ones_like(s))
    return np.stack(F, axis=-1)


def _solve_basis_map(knots):
    """M (8 x 8) with basis = features(s) @ M, s = clamp(inv_h*x+s_bias,0,S)."""
    t3 = float(knots[SPLINE_ORDER])
    h = float(knots[SPLINE_ORDER + 1] - knots[SPLINE_ORDER])
    inv_h = float(1.0 / h)
    s_bias = float(-t3 / h)
    s_max = float(N_GRID)
    g = np.linspace(t3 - 0.5, t3 + N_GRID * h + 0.5, 4001)
    g = np.concatenate([g, np.asarray(knots, np.float64),
                        [t3, t3 + N_GRID * h]])
    sg = np.clip(inv_h * g + s_bias, 0.0, s_max)
    F = _features_f64(sg)
    Bref = _bspline_basis_f64(g, knots)
    M, _, _, _ = np.linalg.lstsq(F, Bref, rcond=None)
    err = np.abs(F @ M - Bref).max()
    # knots come in as float32 and are not exactly uniform, so the closed-form
    # uniform features reproduce the reference basis only to ~1e-7.
    if err > 1e-5:
        raise ValueError(f"basis map residual too large: {err}")
    return M, inv_h, s_bias, s_max


def _prepare_weights(coeff, base_weight, knots):
    M, inv_h, s_bias, s_max = _solve_basis_map(np.asarray(knots, np.float64))
    c64 = np.asarray(coeff, np.float64)                      # (i, o, r)
    # W2[(f, i), o] = sum_r M[f, r] * coeff[i, o, r] for f = 0..6
    w2 = np.einsum("fr,ior->fio", M[:N_FEAT], c64)
    # min-trick cubes are computed negated on device -> negate their weights
    for fi, (k, sgn) in enumerate(CUBES):
        if fi in DVE_CUBES and sgn < 0:
            w2[3 + fi] = -w2[3 + fi]
    # device layout: (ic, fi) blocks of 128 rows, in matmul consumption
    # order, so a single streaming DMA delivers weights just in time
    w2 = w2.reshape(N_FEAT, N_IC, 128, N_OUT).transpose(1, 0, 2, 3)
    w2 = w2.reshape(-1, N_OUT)
    # const feature: one shared bias row = sum_i sum_r M[7,r]*coeff[i,o,r]
    bias_row = np.einsum("r,ior->o", M[N_FEAT], c64)[None, :]
    w2 = np.concatenate([w2, bias_row], axis=0).astype(np.float32)
    w2 = w2.astype(np.float16)
    wb = np.asarray(base_weight, np.float32).astype(np.float16)
    return w2, wb, inv_h, s_bias, s_max


# ----------------------------------------------------------------------------
# Device kernel (one SPMD program, run on all 8 cores)
# ----------------------------------------------------------------------------

def _build_nc(inv_h, s_bias, s_max, repeat=1, loop_n=None):
    """s = clamp(inv_h*x + s_bias, 0, s_max).

    repeat > 1 re-emits the whole compute body (idempotent) for delta-timing.
    loop_n wraps the body in a hardware For_i loop (idempotent) for timing.
    """
    nc = bacc.Bacc()
    x_ext = nc.declare_dram_parameter("x", [B_CORE, N_IN], f32, isOutput=False)
    w2_ext = nc.declare_dram_parameter("w2", [N_W2_ROWS, N_OUT], f16,
                                       isOutput=False)
    wb_ext = nc.declare_dram_parameter("wb", [N_IN, N_OUT], f16, isOutput=False)
    out_ext = nc.declare_dram_parameter("out", [B_CORE, N_OUT], f32, isOutput=True)

    with tile.TileContext(nc) as tc:
        with tc.tile_pool(name="wpool", bufs=1) as wpool, \
             tc.tile_pool(name="xpool", bufs=2) as xpool, \
             tc.tile_pool(name="xtpool", bufs=2) as xtpool, \
             tc.tile_pool(name="fpool", bufs=2) as fpool, \
             tc.tile_pool(name="tpool", bufs=3) as tpool, \
             tc.tile_pool(name="opool", bufs=6) as opool, \
             tc.tile_pool(name="mpool", bufs=1) as mpool, \
             tc.tile_pool(name="psum_t", bufs=1, space="PSUM") as psum_t, \
             tc.tile_pool(name="psum_b", bufs=2, space="PSUM") as psum_b, \
             tc.tile_pool(name="psum_s", bufs=2, space="PSUM") as psum_s:

            ident = mpool.tile([128, 128], f32, tag="ident")
            make_identity(nc, ident[:])
            ones_t = mpool.tile([1, 128], f16, tag="ones")
            nc.vector.memset(ones_t[:], 1.0)

            # per-partition bias constants for ACT: col 0 = s_bias (clamp),
            # cols 1..4 = sgn*(C_CENTER - k) for the cube relus
            biases = mpool.tile([128, 5], f32, tag="biases")
            nc.vector.memset(biases[:, 0:1], float(s_bias))
            for fi, (k, sgn) in enumerate(CUBES):
                nc.vector.memset(biases[:, fi + 1:fi + 2],
                                 float(sgn * (C_CENTER - k)))

            # x window 0 first so transposes can start immediately
            xnat_tiles = {}
            w_off = [sum(WINDOWS[:w]) for w in range(len(WINDOWS))]

            def dma_x(w):
                rows = WINDOWS[w]
                xnat = xpool.tile([128, rows // 128, N_IN], f32, tag="xnat")
                nc.sync.dma_start(
                    out=xnat[:],
                    in_=x_ext[w_off[w]:w_off[w] + rows, :]
                        .rearrange("(j p) i -> p j i", p=128),
                )
                xnat_tiles[w] = xnat

            dma_x(0)

            # resident weights, each as ONE streaming DMA (serial SP-queue
            # dispatch of many small copies would stall early matmuls).
            # w2 HBM layout is already in matmul consumption order.
            wconst = mpool.tile([1, N_OUT], f16, tag="wconst")
            nc.sync.dma_start(out=wconst[:],
                              in_=w2_ext[N_W2_ROWS - 1:N_W2_ROWS, :])
            wball = wpool.tile([128, N_IC, N_OUT], f16, tag="wball")
            nc.sync.dma_start(
                out=wball[:],
                in_=wb_ext.rearrange("(c p) o -> p c o", p=128))
            wb_tiles = {ic: wball[:, ic, :] for ic in range(N_IC)}
            w2all = wpool.tile([128, N_FEAT * N_IC, N_OUT], f16, tag="w2all")
            n_blk = N_FEAT * N_IC
            w2_src = w2_ext[:N_FEAT * N_IN, :].rearrange("(b p) o -> p b o",
                                                         p=128)
            # chunked so early blocks (consumption order) arrive early
            for b0 in range(0, n_blk, 4):
                b1 = min(b0 + 4, n_blk)
                nc.sync.dma_start(out=w2all[:, b0:b1, :],
                                  in_=w2_src[:, b0:b1, :])
            w2_tiles = {}
            for ic in range(N_IC):
                for fi in range(N_FEAT):
                    b = ic * N_FEAT + fi
                    w2_tiles[(fi, ic)] = w2all[:, b, :]

            def transpose_block(w):
                """PE transposes + clamp + base-cast for window w.
                Returns (xth[ic], cfeat[ic])."""
                rows = WINDOWS[w]
                xnat = xnat_tiles[w]
                xth, cf = {}, {}
                for ic in range(N_IC):
                    pt = psum_t.tile([128, rows], f32, tag=f"pt_{ic}")
                    for j in range(rows // 128):
                        nc.tensor.transpose(
                            pt[:, j * 128:(j + 1) * 128],
                            xnat[:, j, ic * 128:(ic + 1) * 128],
                            ident[:],
                        )
                    xt = xtpool.tile([128, rows], f16, tag=f"xth_{ic}")
                    nc.scalar.activation(xt[:], pt[:], AF.Copy)
                    sr = tpool.tile([128, rows], f16, tag="sr")
                    nc.scalar.activation(sr[:], pt[:], AF.Relu,
                                         bias=biases[:, 0:1], scale=inv_h)
                    # c = min(sr, s_max) - C_CENTER  (clamp right + center)
                    sc = fpool.tile([128, rows], f16, tag=f"c_{ic}")
                    nc.vector.tensor_scalar(out=sc[:], in0=sr[:],
                                            scalar1=s_max, scalar2=C_CENTER,
                                            op0=ALU.min, op1=ALU.subtract)
                    xth[ic] = xt
                    cf[ic] = sc
                return xth, cf

            def feature_block(w, cf):
                """Feature chain for window w, balanced across ACT/DVE/Pool.
                Returns feat[(fi,ic)]."""
                rows = WINDOWS[w]
                feat = {}
                for ic in range(N_IC):
                    cc = cf[ic]
                    feat[(0, ic)] = cc
                    c2 = fpool.tile([128, rows], f16, tag=f"c2_{ic}")
                    nc.vector.tensor_tensor(out=c2[:], in0=cc[:], in1=cc[:],
                                            op=ALU.mult)
                    feat[(1, ic)] = c2
                    c3 = fpool.tile([128, rows], f16, tag=f"c3_{ic}")
                    nc.vector.tensor_tensor(out=c3[:], in0=c2[:], in1=cc[:],
                                            op=ALU.mult)
                    feat[(2, ic)] = c3
                    for fi, (k, sgn) in enumerate(CUBES):
                        # u = relu(sgn*(s-k)): DVE tensor_scalar (fastest op,
                        # 352ns measured); sgn<0 via min-trick (w2 negated)
                        u = tpool.tile([128, rows], f16, tag="u")
                        if sgn > 0:  # u = (c - (k-C)) max 0
                            nc.vector.tensor_scalar(
                                out=u[:], in0=cc[:],
                                scalar1=float(k - C_CENTER), scalar2=0.0,
                                op0=ALU.subtract, op1=ALU.max)
                        else:  # m = (c + (C-k)) min 0  (= -relu(k-s))
                            nc.vector.tensor_scalar(
                                out=u[:], in0=cc[:],
                                scalar1=float(C_CENTER - k), scalar2=0.0,
                                op0=ALU.add, op1=ALU.min)
                        u2 = tpool.tile([128, rows], f16, tag="u2")
                        # measured rates: ACT 785, DVE tt 470, Pool tt 1137 ns
                        # -> q1/q3 on ACT, L2/L4 on Pool, rest on DVE
                        if fi in (0, 2):
                            nc.scalar.activation(u2[:], u[:], AF.Square)
                        else:
                            nc.vector.tensor_tensor(out=u2[:], in0=u[:],
                                                    in1=u[:], op=ALU.mult)
                        L = fpool.tile([128, rows], f16, tag=f"L{fi}_{ic}")
                        L_eng = nc.gpsimd if fi in (1, 3) else nc.vector
                        L_eng.tensor_tensor(out=L[:], in0=u2[:], in1=u[:],
                                            op=ALU.mult)
                        feat[(3 + fi, ic)] = L
                return feat

            def matmul_block(w, xth, feat):
                rows = WINDOWS[w]
                b0 = w_off[w]
                last = w == len(WINDOWS) - 1
                for bt in range(rows // 128):
                    # final batch tile: split matmuls into 128-col pieces so
                    # the in-flight PE queue drains fast and the stop
                    # semaphore (gating the last epilogue) posts sooner
                    # NOTE: column-split PSUM groups are illegal (start bit
                    # zeroes the whole PSUM region) -- keep csplit == 1
                    csplit = 1
                    cw = N_OUT // csplit
                    bs = slice(bt * 128, (bt + 1) * 128)
                    acc_b = psum_b.tile([128, N_OUT], f32, tag="accb")
                    for ic in range(N_IC):
                        nc.tensor.matmul(
                            acc_b[:], xth[ic][:, bs], wb_tiles[ic],
                            start=(ic == 0), stop=(ic == N_IC - 1),
                        )
                    acc_s = psum_s.tile([128, N_OUT], f32, tag="accs")
                    n_chunks = (N_FEAT * N_IC + 1) * csplit
                    ci = 0
                    for cs in range(csplit):
                        co = slice(cs * cw, (cs + 1) * cw)
                        nc.tensor.matmul(acc_s[:, co], ones_t[:],
                                         wconst[:, co],
                                         start=True, stop=False)
                        ci += 1
                    # ic-major: consume features in production order
                    for ic in range(N_IC):
                        for fi in range(N_FEAT):
                            for cs in range(csplit):
                                co = slice(cs * cw, (cs + 1) * cw)
                                nc.tensor.matmul(
                                    acc_s[:, co], feat[(fi, ic)][:, bs],
                                    w2_tiles[(fi, ic)][:, co],
                                    start=False, stop=(ci >= n_chunks - csplit),
                                )
                                ci += 1
                    stag, otag = ("silu", "out") if not last else (
                        f"siluL_{bt}", f"outL_{bt}")
                    bfs = None if not last else 1
                    silu_t = opool.tile([128, N_OUT], f32, tag=stag, bufs=bfs)
                    nc.scalar.activation(silu_t[:], acc_b[:], AF.Silu)
                    out_t = opool.tile([128, N_OUT], f32, tag=otag, bufs=bfs)
                    # last window: DVE (faster op) shortens the drain tail
                    eng = nc.vector if last else (
                        nc.gpsimd if EPI_ENGINE == "pool" else nc.vector)
                    eng.tensor_tensor(out=out_t[:], in0=acc_s[:],
                                      in1=silu_t[:], op=ALU.add)
                    nc.sync.dma_start(
                        out=out_ext[b0 + bt * 128:b0 + (bt + 1) * 128, :],
                        in_=out_t[:],
                    )

            import contextlib
            loop_cm = (tc.For_i(0, loop_n, 1) if loop_n
                       else contextlib.nullcontext())
            n_win = len(WINDOWS)
            with loop_cm:
                for _ in range(repeat):
                    xth, cf = transpose_block(0)
                    for w in range(n_win):
                        feat = feature_block(w, cf)
                        if w + 1 < n_win:
                            dma_x(w + 1)
                            xth_n, cf_n = transpose_block(w + 1)
                        matmul_block(w, xth, feat)
                        if w + 1 < n_win:
                            xth, cf = xth_n, cf_n
                    # next repeat reuses window 0: re-issue dma + transpose
                    if repeat > 1 or loop_n:
                        dma_x(0)
    nc.compile()
    return nc


_NC_CACHE = {}


def _get_nc(inv_h, s_bias, s_max):
    key = (inv_h, s_bias, s_max)
    if key not in _NC_CACHE:
        _NC_CACHE[key] = _build_nc(inv_h, s_bias, s_max)
    return _NC_CACHE[key]


def kernel(x, coeff, base_weight, knots):
    x = np.asarray(x, np.float32)
    assert x.shape == (BATCH, N_IN), x.shape
    w2, wb, inv_h, s_bias, s_max = _prepare_weights(coeff, base_weight, knots)
    nc = _get_nc(inv_h, s_bias, s_max)

    in_maps = []
    for c in range(N_CORES):
        in_maps.append({
            "x": x[c * B_CORE:(c + 1) * B_CORE],
            "w2": w2,
            "wb": wb,
        })
    last_err = None
    for attempt in range(3):
        try:
            results = run_bass_kernel_spmd(
                nc, in_maps, list(range(N_CORES))).results
            break
        except Exception as e:  # transient device wedge: retry
            last_err = e
            if attempt == 2:
                raise
            import time
            time.sleep(2.0)
    out = np.concatenate([results[c]["out"] for c in range(N_CORES)], axis=0)
    return out.astype(np.float32)
